# revision 10
# baseline (speedup 1.0000x reference)
"""Trainium2 Bass kernel for nn_EnhancedLocalAttention.

Reference semantics (B=4, L=4096, C=1024, H=16, D=64, WIN=256, step=128):
  qkv = x @ W_qkv + b_qkv -> q,k,v [B,H,L,D]
  overlapping windows n: tokens [n*128, n*128+256)
  per (b,h,n): S = (Q_win^T K_win)/8  (D x D, contracted over the 256 window
  tokens), P = softmax(S, axis=-1), O = P @ V_win^T  (D x W)
  regroup: rows of reshape(O, [256, 64]) laid at tokens n*256..n*256+255,
  slice to L -> only windows 0..15 survive; then @ W_out + b_out.

Sharding: 8 cores = (4 batches) x (2 window-halves of 8 windows each).
Each core consumes 9 x 128-token chunks and produces 2048 output rows.

v6 pipeline (round r):
  QKV(r) + S-phase of window r-2 + P^T/O/out-proj phase of window r-3.
The softmax chain (EXP -> rowsum -> recip) for a window runs a full round
before its results feed matmuls, so the PE never waits on it.
  - host pre-casts x/W to f16
  - x^T via ONE DMA xbar transpose per chunk straight from DRAM
  - V^T via W-stationary matmuls; P^T via identity-matmuls (tile_position)
  - S for 4 head-pairs batched per PSUM bank; EXPs batched 4-wide (3D APs)
  - PSUM: pool A = 4 banks ping-ponging Q/K accumulators, pool B = 4 banks
    for everything else, allocation order chosen so every slot reuse is
    gated by a prompt copy (no accumulation convoys)
"""

import threading

import numpy as np

import concourse.bacc as bacc
import concourse.masks as masks
import concourse.mybir as mybir
import concourse.tile as tile
from concourse._compat import get_trn_type
from concourse.bass_utils import run_bass_kernel_spmd

F32 = mybir.dt.float32
F16 = mybir.dt.float16
EXP = mybir.ActivationFunctionType.Exp
AXX = mybir.AxisListType.X

B, L, C = 4, 4096, 1024
H, D, WIN, STEP = 16, 64, 256, 128
NCHUNK = 9            # 128-token chunks per core
NWIN = 8              # windows per core
TOK = NCHUNK * 128    # 1152 input tokens per core
OUT_ROWS = NWIN * 256 # 2048 output rows per core


def interleave(a, b):
    """Merge two unit lists proportionally (Bresenham)."""
    if not b:
        return list(a)
    if not a:
        return list(b)
    out = []
    ia = ib = 0
    while ia < len(a) or ib < len(b):
        if ib >= len(b) or (ia < len(a) and ia * len(b) <= ib * len(a)):
            out.append(a[ia]); ia += 1
        else:
            out.append(b[ib]); ib += 1
    return out


def build_program(with_bias=False):
    nc = bacc.Bacc(
        get_trn_type() or "TRN2",
        target_bir_lowering=False,
        debug=False,
        num_devices=8,
    )
    xs = nc.dram_tensor("xs", [TOK, C], F16, kind="ExternalInput")
    wqkv = nc.dram_tensor("wqkv", [C, 3 * C], F16, kind="ExternalInput")
    bqkv = nc.dram_tensor("bqkv", [3 * C], F32, kind="ExternalInput")
    wout = nc.dram_tensor("wout", [C, C], F16, kind="ExternalInput")
    bout = nc.dram_tensor("bout", [C], F32, kind="ExternalInput")
    out = nc.dram_tensor("out", [OUT_ROWS, C], F32, kind="ExternalOutput")

    from contextlib import ExitStack

    with tile.TileContext(nc) as tc, ExitStack() as ctx:
        pool = lambda name, bufs: ctx.enter_context(tc.tile_pool(name=name, bufs=bufs))
        wq_pool = pool("wq", 8)
        wv_pool = pool("wv", 8)
        wo_pool = pool("wo", 8)
        const_pool = pool("const", 1)
        xt_pool = pool("xt", 5)
        q_pool = pool("q", 5)
        k_pool = pool("k", 5)
        vt_pool = pool("vt", 6)
        at_pool = pool("at", 8)
        st_pool = pool("st", 8)
        yt_pool = pool("yt", 8)
        o_pool = pool("o", 3)
        ps_a = ctx.enter_context(tc.tile_pool(name="psa", bufs=4, space="PSUM"))
        ps_b = ctx.enter_context(tc.tile_pool(name="psb", bufs=4, space="PSUM"))

        # --- constants / weights ---
        idf16 = const_pool.tile([128, 128], F16, tag="idf16", name="idf16")
        masks.make_identity(nc, idf16[:])
        ones = const_pool.tile([1, 128], F16, tag="ones", name="ones")
        nc.vector.memset(ones[:], 1.0)
        bq_sb = const_pool.tile([1, 3 * C], F16, tag="bq", name="bq_sb")
        bo_sb = const_pool.tile([1, C], F16, tag="bo", name="bo_sb")
        if with_bias:
            nc.gpsimd.dma_start(bq_sb[:], bqkv.ap().rearrange("(a f) -> a f", a=1))
            nc.gpsimd.dma_start(bo_sb[:], bout.ap().rearrange("(a f) -> a f", a=1))

        # x^T per chunk as one [128, 8*128] tile; block cb = cols cb*128..+128
        xt_all = [None] * NCHUNK

        def prefetch_xt(r):
            xtt = xt_pool.tile([128, C], F16, tag="xt", name="xtt")
            nc.sync.dma_start(
                xtt[:].rearrange("p (b t) -> p b t", b=8),
                xs.ap()[r * 128 : (r + 1) * 128, :],
                transpose=True,
            )
            xt_all[r] = xtt

        prefetch_xt(0)
        prefetch_xt(1)
        prefetch_xt(2)

        # weights stream on two DMA queues (even blocks gpsimd, odd scalar)
        def weng(cb):
            return nc.gpsimd if cb % 2 == 0 else nc.scalar

        wqa_sb = []
        for cb in range(8):
            t = wq_pool.tile([128, 2 * C], F16, tag="wqa", name=f"wqa{cb}")
            weng(cb).dma_start(t[:], wqkv.ap()[cb * 128 : (cb + 1) * 128, 0 : 2 * C])
            wqa_sb.append(t)
        wv_sb = []
        for cb in range(8):
            t = wv_pool.tile([128, C], F16, tag="wv", name=f"wv{cb}")
            weng(cb).dma_start(
                t[:], wqkv.ap()[cb * 128 : (cb + 1) * 128, 2 * C : 3 * C]
            )
            wv_sb.append(t)
        wo_sb = []
        for cb in range(8):
            t = wo_pool.tile([128, C], F16, tag="wo", name=f"wo{cb}")
            weng(cb).dma_start(t[:], wout.ap()[cb * 128 : (cb + 1) * 128, :])
            wo_sb.append(t)

        q_sb = [None] * NCHUNK
        k_sb = [None] * NCHUNK
        vt_sb = [None] * NCHUNK   # [e-pair 128, hp*128 + tok]
        wstate = [
            {"pe4": [None, None], "rs4": [None, None]} for _ in range(NWIN)
        ]

        def qkv_units(r):
            """Chunk r: Q,K (pool A) and V^T (pool B) projections."""
            st = {}

            def u_pref():
                if r + 3 < NCHUNK:
                    prefetch_xt(r + 3)
                st["xt"] = [
                    xt_all[r][:, cb * 128 : (cb + 1) * 128] for cb in range(8)
                ]

            def u_qk_alloc():
                st["pq"] = [
                    ps_a.tile([128, 512], F32, tag="a", name=f"pq{i}")
                    for i in range(4)
                ]

            def u_qk(cb):
                def f():
                    for i in range(4):
                        nc.tensor.matmul(
                            st["pq"][i][:],
                            st["xt"][cb],
                            wqa_sb[cb][:, i * 512 : (i + 1) * 512],
                            start=(cb == 0),
                            stop=(not with_bias and cb == 7),
                        )
                return f

            def u_qk_fin():
                if with_bias:
                    for i in range(4):
                        nc.tensor.matmul(
                            st["pq"][i][:],
                            ones[:, :],
                            bq_sb[:, i * 512 : (i + 1) * 512],
                            start=False,
                            stop=True,
                        )
                qt = q_pool.tile([128, C], F16, tag="q", name="qt")
                nc.scalar.mul(qt[:, 0:512], st["pq"][0][:], 0.125)
                nc.scalar.mul(qt[:, 512:1024], st["pq"][1][:], 0.125)
                q_sb[r] = qt
                kt = k_pool.tile([128, C], F16, tag="k", name="kt")
                nc.vector.tensor_copy(kt[:, 0:512], st["pq"][2][:])
                nc.vector.tensor_copy(kt[:, 512:1024], st["pq"][3][:])
                k_sb[r] = kt

            def u_v_alloc():
                st["pv"] = [
                    ps_b.tile([128, 512], F32, tag="b", name=f"pv{i}")
                    for i in range(2)
                ]

            def u_v(hp):
                def f():
                    pv = st["pv"][hp // 4]
                    sl = (hp % 4) * 128
                    for cb in range(8):
                        nc.tensor.matmul(
                            pv[:, sl : sl + 128],
                            wv_sb[cb][:, hp * 128 : (hp + 1) * 128],
                            st["xt"][cb],
                            start=(cb == 0),
                            stop=(not with_bias and cb == 7),
                        )
                    if with_bias:
                        nc.tensor.matmul(
                            pv[:, sl : sl + 128],
                            bq_sb[:, 2 * C + hp * 128 : 2 * C + (hp + 1) * 128],
                            ones[:, :],
                            start=False,
                            stop=True,
                        )
                return f

            def u_v_fin():
                v_t = vt_pool.tile([128, C], F16, tag="vt", name="v_t")
                nc.vector.tensor_copy(v_t[:, 0:512], st["pv"][0][:])
                nc.scalar.copy(v_t[:, 512:1024], st["pv"][1][:])
                vt_sb[r] = v_t

            units = [u_pref, u_qk_alloc]
            units += [u_qk(cb) for cb in range(8)]
            units += [u_qk_fin, u_v_alloc]
            units += [u_v(hp) for hp in range(8)]
            units += [u_v_fin]
            return units

        def sphase_units(w):
            """Window w scores: S matmuls (4 head-pairs per PSUM bank),
            batched EXP / rowsum / reciprocal. Results land in SBUF for
            next round's phase2."""
            ws = wstate[w]

            def u_sb(j):
                def f():
                    sbt = ps_b.tile([128, 512], F32, tag="b", name="sbt")
                    for ii in range(4):
                        hp = 4 * j + ii
                        s = sbt[:, ii * 128 : (ii + 1) * 128]
                        for rr, (b0, b1) in (
                            (w, (True, False)),
                            (w + 1, (False, True)),
                        ):
                            nc.tensor.matmul(
                                s,
                                q_sb[rr][:, hp * 128 : (hp + 1) * 128],
                                k_sb[rr][:, hp * 128 : (hp + 1) * 128],
                                start=b0,
                                stop=b1,
                            )
                    pe4 = at_pool.tile([128, 256], F16, tag="pe4", name="pe4")
                    sb3 = sbt[:].rearrange("p (h c) -> p h c", h=4)
                    pe3 = pe4[:].rearrange("p (h e) -> p h e", h=4)
                    nc.scalar.activation(pe3[0:64], sb3[0:64, :, 0:64], EXP)
                    nc.scalar.activation(pe3[64:128], sb3[64:128, :, 64:128], EXP)
                    ssum4 = st_pool.tile([128, 4], F32, tag="ssum", name="ssum4")
                    nc.vector.reduce_sum(ssum4[:], pe3, axis=AXX)
                    rs4 = st_pool.tile([128, 4], F32, tag="rs", name="rs4")
                    nc.vector.reciprocal(rs4[:], ssum4[:])
                    ws["pe4"][j] = pe4
                    ws["rs4"][j] = rs4
                return f

            return [u_sb(0), u_sb(1)]

        def phase2_units(w):
            """Window w: normalize, P^T, O, out-projection (chain results
            from last round's S-phase)."""
            ws = wstate[w]
            yt2 = [None] * 4
            ptsb = [None] * 4
            ptw = [None] * 2
            yw = [None] * 4

            def u_pt(pp):
                def f():
                    if pp % 2 == 0:
                        ptw[pp // 2] = ps_b.tile(
                            [128, 512], F32, tag="b", name="ptw"
                        )
                    ptp2 = ptw[pp // 2][:, (pp % 2) * 128 : (pp % 2) * 128 + 128]
                    j, pe4, rs4 = pp // 2, ws["pe4"][pp // 2], ws["rs4"][pp // 2]
                    for i in (0, 1):
                        hp = 2 * pp + i
                        ii = hp % 4
                        p_n = at_pool.tile([128, 64], F16, tag="p_n", name="p_n")
                        nc.vector.tensor_scalar_mul(
                            p_n[:],
                            pe4[:, ii * 64 : (ii + 1) * 64],
                            rs4[:, ii : ii + 1],
                        )
                        nc.tensor.matmul(
                            ptp2[0:64, i * 64 : (i + 1) * 64],
                            p_n[0:64, :],
                            idf16[0:64, 0:64],
                            start=True,
                            stop=True,
                            tile_position=(0, 0),
                        )
                        nc.tensor.matmul(
                            ptp2[64:128, i * 64 : (i + 1) * 64],
                            p_n[64:128, :],
                            idf16[64:128, 64:128],
                            start=True,
                            stop=True,
                            tile_position=(64, 64),
                        )
                    pt2 = at_pool.tile([128, 128], F16, tag="ptsb", name="pt2")
                    eng = nc.vector.tensor_copy if pp % 2 else nc.scalar.copy
                    eng(pt2[:], ptp2)
                    ptsb[pp] = pt2
                return f

            def u_o(hp):
                def f():
                    if hp % 2 == 0:
                        yw[hp // 2] = ps_b.tile(
                            [128, 512], F32, tag="b", name="yw"
                        )
                    ypsum = yw[hp // 2][:, (hp % 2) * 256 : (hp % 2) * 256 + 256]
                    pt2 = ptsb[hp // 2]
                    c0 = (hp % 2) * 64
                    for po in (0, 64):
                        rh = pt2[po : po + 64, c0 : c0 + 64]
                        for wq in range(4):
                            vtt = vt_sb[w + wq // 2]
                            col = hp * 128 + (wq % 2) * 64
                            nc.tensor.matmul(
                                ypsum[po : po + 64, wq * 64 : (wq + 1) * 64],
                                vtt[po : po + 64, col : col + 64],
                                rh,
                                start=True,
                                stop=True,
                                tile_position=(po, po),
                            )
                    if hp % 2 == 1:
                        # Y^T[c, g*256 + d*4+wq] = yw[c, g*256 + wq*64+d]
                        ytt = yt_pool.tile([128, 512], F16, tag="yt", name="ytt")
                        eng = (
                            nc.vector.tensor_copy
                            if (hp // 2) % 2
                            else nc.scalar.copy
                        )
                        eng(
                            ytt[:].rearrange("p (g b a) -> p g a b", g=2, a=4),
                            yw[hp // 2][:].rearrange(
                                "p (g a b) -> p g a b", g=2, a=4
                            ),
                        )
                        yt2[hp // 2] = ytt
                return f

            def u_op(th):
                def f():
                    po_m = [
                        ps_a.tile([128, 512], F32, tag="a", name=f"pom{i}")
                        for i in range(2)
                    ]
                    for cb in range(8):
                        lh = yt2[cb // 2][
                            :, (cb % 2) * 256 + th * 128 : (cb % 2) * 256 + th * 128 + 128
                        ]
                        for mi in range(2):
                            nc.tensor.matmul(
                                po_m[mi][:],
                                lh,
                                wo_sb[cb][:, mi * 512 : (mi + 1) * 512],
                                start=(cb == 0),
                                stop=(not with_bias and cb == 7),
                            )
                    if with_bias:
                        for mi in range(2):
                            nc.tensor.matmul(
                                po_m[mi][:],
                                ones[:, :],
                                bo_sb[:, mi * 512 : (mi + 1) * 512],
                                start=False,
                                stop=True,
                            )
                    ot = o_pool.tile([128, C], F32, tag="o", name="ot")
                    nc.vector.tensor_copy(ot[:, 0:512], po_m[0][:])
                    nc.scalar.copy(ot[:, 512:1024], po_m[1][:])
                    row = w * 256 + th * 128
                    nc.sync.dma_start(out.ap()[row : row + 128, :], ot[:])
                return f

            return [
                u_pt(0), u_pt(1), u_o(0), u_o(1), u_o(2), u_o(3),
                u_pt(2), u_pt(3), u_o(4), u_o(5), u_o(6), u_o(7),
                u_op(0), u_op(1),
            ]

        for r in range(NCHUNK + 1):
            qk = qkv_units(r) if r < NCHUNK else []
            win = []
            if 2 <= r < NWIN + 2:
                win += sphase_units(r - 2)
            if 3 <= r < NWIN + 2:
                win += phase2_units(r - 3)
            if r == NWIN + 1:
                # last window's phase2 rides the same round, hidden
                # behind phase2(NWIN-2)
                win += phase2_units(NWIN - 1)
            # S-phase first (its deps are ready), then phase2 of the
            # previous window
            for u in interleave(qk, win):
                u()

    nc.compile()
    return nc


_CACHE = {}
_LOCK = threading.Lock()


def _get_program(with_bias=False):
    key = f"nc_bias{with_bias}"
    with _LOCK:
        if key not in _CACHE:
            _CACHE[key] = build_program(with_bias=with_bias)
        return _CACHE[key]


def make_in_maps(x, W_qkv, b_qkv, W_out, b_out):
    x16 = np.asarray(x, dtype=np.float16)
    wqkv16 = np.asarray(W_qkv, dtype=np.float16)
    wout16 = np.asarray(W_out, dtype=np.float16)
    bqkv = np.asarray(b_qkv, dtype=np.float32)
    bout = np.asarray(b_out, dtype=np.float32)
    in_maps = []
    for cid in range(8):
        b, half = cid // 2, cid % 2
        t0 = half * NWIN * STEP
        in_maps.append(
            {
                "xs": np.ascontiguousarray(x16[b, t0 : t0 + TOK, :]),
                "wqkv": wqkv16,
                "bqkv": bqkv,
                "wout": wout16,
                "bout": bout,
            }
        )
    return in_maps


def kernel(x, W_qkv, b_qkv, W_out, b_out):
    with_bias = bool(np.any(b_qkv)) or bool(np.any(b_out))
    nc = _get_program(with_bias=with_bias)
    in_maps = make_in_maps(x, W_qkv, b_qkv, W_out, b_out)
    res = run_bass_kernel_spmd(nc, in_maps, core_ids=list(range(8)))
    out_full = np.empty((B, L, C), dtype=np.float32)
    for cid in range(8):
        b, half = cid // 2, cid % 2
        out_full[b, half * OUT_ROWS : (half + 1) * OUT_ROWS, :] = res.results[cid][
            "out"
        ]
    return out_full


# revision 12
# speedup vs baseline: 1.1397x; 1.1397x over previous
"""Trainium2 Bass kernel for nn_EnhancedLocalAttention.

Reference semantics (B=4, L=4096, C=1024, H=16, D=64, WIN=256, step=128):
  qkv = x @ W_qkv + b_qkv -> q,k,v [B,H,L,D]
  overlapping windows n: tokens [n*128, n*128+256)
  per (b,h,n): S = (Q_win^T K_win)/8  (D x D, contracted over the 256 window
  tokens), P = softmax(S, axis=-1), O = P @ V_win^T  (D x W)
  regroup: rows of reshape(O, [256, 64]) laid at tokens n*256..n*256+255,
  slice to L -> only windows 0..15 survive; then @ W_out + b_out.

Sharding: 8 cores = (4 batches) x (2 window-halves of 8 windows each).
Each core consumes 9 x 128-token chunks and produces 2048 output rows.

v6 pipeline (round r):
  QKV(r) + S-phase of window r-2 + P^T/O/out-proj phase of window r-3.
The softmax chain (EXP -> rowsum -> recip) for a window runs a full round
before its results feed matmuls, so the PE never waits on it.
  - host pre-casts x/W to f16
  - x^T via ONE DMA xbar transpose per chunk straight from DRAM
  - V^T via W-stationary matmuls; P^T via identity-matmuls (tile_position)
  - S for 4 head-pairs batched per PSUM bank; EXPs batched 4-wide (3D APs)
  - PSUM: pool A = 4 banks ping-ponging Q/K accumulators, pool B = 4 banks
    for everything else, allocation order chosen so every slot reuse is
    gated by a prompt copy (no accumulation convoys)
"""

import threading

import numpy as np

import concourse.bacc as bacc
import concourse.masks as masks
import concourse.mybir as mybir
import concourse.tile as tile
from concourse._compat import get_trn_type
from concourse.bass_utils import run_bass_kernel_spmd

F32 = mybir.dt.float32
F16 = mybir.dt.float16
EXP = mybir.ActivationFunctionType.Exp
AXX = mybir.AxisListType.X

B, L, C = 4, 4096, 1024
H, D, WIN, STEP = 16, 64, 256, 128
NCHUNK = 9            # 128-token chunks per core
NWIN = 8              # windows per core
TOK = NCHUNK * 128    # 1152 input tokens per core
OUT_ROWS = NWIN * 256 # 2048 output rows per core


def interleave(a, b):
    """Merge two unit lists proportionally (Bresenham)."""
    if not b:
        return list(a)
    if not a:
        return list(b)
    out = []
    ia = ib = 0
    while ia < len(a) or ib < len(b):
        if ib >= len(b) or (ia < len(a) and ia * len(b) <= ib * len(a)):
            out.append(a[ia]); ia += 1
        else:
            out.append(b[ib]); ib += 1
    return out


def build_program(with_bias=False):
    nc = bacc.Bacc(
        get_trn_type() or "TRN2",
        target_bir_lowering=False,
        debug=False,
        num_devices=8,
    )
    xs = nc.dram_tensor("xs", [TOK, C], F16, kind="ExternalInput")
    wqkv = nc.dram_tensor("wqkv", [C, 3 * C], F16, kind="ExternalInput")
    bqkv = nc.dram_tensor("bqkv", [3 * C], F32, kind="ExternalInput")
    wout = nc.dram_tensor("wout", [C, C], F16, kind="ExternalInput")
    bout = nc.dram_tensor("bout", [C], F32, kind="ExternalInput")
    out = nc.dram_tensor("out", [OUT_ROWS, C], F32, kind="ExternalOutput")

    from contextlib import ExitStack

    with tile.TileContext(nc) as tc, ExitStack() as ctx:
        pool = lambda name, bufs: ctx.enter_context(tc.tile_pool(name=name, bufs=bufs))
        wq_pool = pool("wq", 8)
        wv_pool = pool("wv", 8)
        wo_pool = pool("wo", 8)
        const_pool = pool("const", 1)
        xt_pool = pool("xt", 5)
        q_pool = pool("q", 5)
        k_pool = pool("k", 5)
        vt_pool = pool("vt", 6)
        at_pool = pool("at", 8)
        st_pool = pool("st", 8)
        yt_pool = pool("yt", 8)
        o_pool = pool("o", 3)
        ps_a = ctx.enter_context(tc.tile_pool(name="psa", bufs=4, space="PSUM"))
        ps_b = ctx.enter_context(tc.tile_pool(name="psb", bufs=4, space="PSUM"))

        # --- constants / weights ---
        idf16 = const_pool.tile([128, 128], F16, tag="idf16", name="idf16")
        masks.make_identity(nc, idf16[:])
        ones = const_pool.tile([1, 128], F16, tag="ones", name="ones")
        nc.vector.memset(ones[:], 1.0)
        bq_sb = const_pool.tile([1, 3 * C], F16, tag="bq", name="bq_sb")
        bo_sb = const_pool.tile([1, C], F16, tag="bo", name="bo_sb")
        if with_bias:
            nc.gpsimd.dma_start(bq_sb[:], bqkv.ap().rearrange("(a f) -> a f", a=1))
            nc.gpsimd.dma_start(bo_sb[:], bout.ap().rearrange("(a f) -> a f", a=1))

        # x^T per chunk as one [128, 8*128] tile; block cb = cols cb*128..+128
        xt_all = [None] * NCHUNK

        def prefetch_xt(r):
            xtt = xt_pool.tile([128, C], F16, tag="xt", name="xtt")
            nc.sync.dma_start(
                xtt[:].rearrange("p (b t) -> p b t", b=8),
                xs.ap()[r * 128 : (r + 1) * 128, :],
                transpose=True,
            )
            xt_all[r] = xtt

        prefetch_xt(0)
        prefetch_xt(1)
        prefetch_xt(2)

        wqa_sb = []
        for cb in range(8):
            t = wq_pool.tile([128, 2 * C], F16, tag="wqa", name=f"wqa{cb}")
            nc.gpsimd.dma_start(t[:], wqkv.ap()[cb * 128 : (cb + 1) * 128, 0 : 2 * C])
            wqa_sb.append(t)
        wv_sb = []
        for cb in range(8):
            t = wv_pool.tile([128, C], F16, tag="wv", name=f"wv{cb}")
            nc.gpsimd.dma_start(
                t[:], wqkv.ap()[cb * 128 : (cb + 1) * 128, 2 * C : 3 * C]
            )
            wv_sb.append(t)
        wo_sb = []
        for cb in range(8):
            t = wo_pool.tile([128, C], F16, tag="wo", name=f"wo{cb}")
            nc.gpsimd.dma_start(t[:], wout.ap()[cb * 128 : (cb + 1) * 128, :])
            wo_sb.append(t)

        q_sb = [None] * NCHUNK
        k_sb = [None] * NCHUNK
        vt_sb = [None] * NCHUNK   # [e-pair 128, hp*128 + tok]
        wstate = [
            {"pe4": [None, None], "rs4": [None, None]} for _ in range(NWIN)
        ]

        def qkv_units(r):
            """Chunk r: Q,K (pool A) and V^T (pool B) projections."""
            st = {}

            def u_pref():
                if r + 3 < NCHUNK:
                    prefetch_xt(r + 3)
                st["xt"] = [
                    xt_all[r][:, cb * 128 : (cb + 1) * 128] for cb in range(8)
                ]

            def u_qk_alloc():
                st["pq"] = [
                    ps_a.tile([128, 512], F32, tag="a", name=f"pq{i}")
                    for i in range(4)
                ]

            def u_qk(cb):
                def f():
                    for i in range(4):
                        nc.tensor.matmul(
                            st["pq"][i][:],
                            st["xt"][cb],
                            wqa_sb[cb][:, i * 512 : (i + 1) * 512],
                            start=(cb == 0),
                            stop=(not with_bias and cb == 7),
                        )
                return f

            def u_qk_fin():
                if with_bias:
                    for i in range(4):
                        nc.tensor.matmul(
                            st["pq"][i][:],
                            ones[:, :],
                            bq_sb[:, i * 512 : (i + 1) * 512],
                            start=False,
                            stop=True,
                        )
                qt = q_pool.tile([128, C], F16, tag="q", name="qt")
                nc.scalar.mul(qt[:, 0:512], st["pq"][0][:], 0.125)
                nc.scalar.mul(qt[:, 512:1024], st["pq"][1][:], 0.125)
                q_sb[r] = qt
                kt = k_pool.tile([128, C], F16, tag="k", name="kt")
                nc.vector.tensor_copy(kt[:, 0:512], st["pq"][2][:])
                nc.vector.tensor_copy(kt[:, 512:1024], st["pq"][3][:])
                k_sb[r] = kt

            def u_v_alloc():
                st["pv"] = [
                    ps_b.tile([128, 512], F32, tag="b", name=f"pv{i}")
                    for i in range(2)
                ]

            def u_v(hp):
                def f():
                    pv = st["pv"][hp // 4]
                    sl = (hp % 4) * 128
                    for cb in range(8):
                        nc.tensor.matmul(
                            pv[:, sl : sl + 128],
                            wv_sb[cb][:, hp * 128 : (hp + 1) * 128],
                            st["xt"][cb],
                            start=(cb == 0),
                            stop=(not with_bias and cb == 7),
                        )
                    if with_bias:
                        nc.tensor.matmul(
                            pv[:, sl : sl + 128],
                            bq_sb[:, 2 * C + hp * 128 : 2 * C + (hp + 1) * 128],
                            ones[:, :],
                            start=False,
                            stop=True,
                        )
                return f

            def u_v_fin():
                v_t = vt_pool.tile([128, C], F16, tag="vt", name="v_t")
                nc.vector.tensor_copy(v_t[:, 0:512], st["pv"][0][:])
                nc.scalar.copy(v_t[:, 512:1024], st["pv"][1][:])
                vt_sb[r] = v_t

            units = [u_pref, u_qk_alloc]
            units += [u_qk(cb) for cb in range(8)]
            units += [u_qk_fin, u_v_alloc]
            units += [u_v(hp) for hp in range(8)]
            units += [u_v_fin]
            return units

        def sphase_units(w):
            """Window w scores: S matmuls (4 head-pairs per PSUM bank),
            batched EXP / rowsum / reciprocal. Results land in SBUF for
            next round's phase2."""
            ws = wstate[w]

            def u_sb(j):
                def f():
                    sbt = ps_b.tile([128, 512], F32, tag="b", name="sbt")
                    for ii in range(4):
                        hp = 4 * j + ii
                        s = sbt[:, ii * 128 : (ii + 1) * 128]
                        for rr, (b0, b1) in (
                            (w, (True, False)),
                            (w + 1, (False, True)),
                        ):
                            nc.tensor.matmul(
                                s,
                                q_sb[rr][:, hp * 128 : (hp + 1) * 128],
                                k_sb[rr][:, hp * 128 : (hp + 1) * 128],
                                start=b0,
                                stop=b1,
                            )
                    pe4 = at_pool.tile([128, 256], F16, tag="pe4", name="pe4")
                    sb3 = sbt[:].rearrange("p (h c) -> p h c", h=4)
                    pe3 = pe4[:].rearrange("p (h e) -> p h e", h=4)
                    nc.scalar.activation(pe3[0:64], sb3[0:64, :, 0:64], EXP)
                    nc.scalar.activation(pe3[64:128], sb3[64:128, :, 64:128], EXP)
                    ssum4 = st_pool.tile([128, 4], F32, tag="ssum", name="ssum4")
                    nc.vector.reduce_sum(ssum4[:], pe3, axis=AXX)
                    rs4 = st_pool.tile([128, 4], F32, tag="rs", name="rs4")
                    nc.vector.reciprocal(rs4[:], ssum4[:])
                    ws["pe4"][j] = pe4
                    ws["rs4"][j] = rs4
                return f

            return [u_sb(0), u_sb(1)]

        def phase2_units(w):
            """Window w: normalize, P^T, O, out-projection (chain results
            from last round's S-phase)."""
            ws = wstate[w]
            yt2 = [None] * 4
            ptsb = [None] * 4
            ptw = [None] * 2
            yw = [None] * 4

            def u_pt(pp):
                def f():
                    if pp % 2 == 0:
                        ptw[pp // 2] = ps_b.tile(
                            [128, 512], F32, tag="b", name="ptw"
                        )
                    ptp2 = ptw[pp // 2][:, (pp % 2) * 128 : (pp % 2) * 128 + 128]
                    j, pe4, rs4 = pp // 2, ws["pe4"][pp // 2], ws["rs4"][pp // 2]
                    for i in (0, 1):
                        hp = 2 * pp + i
                        ii = hp % 4
                        p_n = at_pool.tile([128, 64], F16, tag="p_n", name="p_n")
                        nc.vector.tensor_scalar_mul(
                            p_n[:],
                            pe4[:, ii * 64 : (ii + 1) * 64],
                            rs4[:, ii : ii + 1],
                        )
                        nc.tensor.matmul(
                            ptp2[0:64, i * 64 : (i + 1) * 64],
                            p_n[0:64, :],
                            idf16[0:64, 0:64],
                            start=True,
                            stop=True,
                            tile_position=(0, 0),
                        )
                        nc.tensor.matmul(
                            ptp2[64:128, i * 64 : (i + 1) * 64],
                            p_n[64:128, :],
                            idf16[64:128, 64:128],
                            start=True,
                            stop=True,
                            tile_position=(64, 64),
                        )
                    pt2 = at_pool.tile([128, 128], F16, tag="ptsb", name="pt2")
                    eng = nc.vector.tensor_copy if pp % 2 else nc.scalar.copy
                    eng(pt2[:], ptp2)
                    ptsb[pp] = pt2
                return f

            def u_o(hp):
                def f():
                    if hp % 2 == 0:
                        yw[hp // 2] = ps_b.tile(
                            [128, 512], F32, tag="b", name="yw"
                        )
                    ypsum = yw[hp // 2][:, (hp % 2) * 256 : (hp % 2) * 256 + 256]
                    pt2 = ptsb[hp // 2]
                    c0 = (hp % 2) * 64
                    for po in (0, 64):
                        rh = pt2[po : po + 64, c0 : c0 + 64]
                        for wq in range(4):
                            vtt = vt_sb[w + wq // 2]
                            col = hp * 128 + (wq % 2) * 64
                            nc.tensor.matmul(
                                ypsum[po : po + 64, wq * 64 : (wq + 1) * 64],
                                vtt[po : po + 64, col : col + 64],
                                rh,
                                start=True,
                                stop=True,
                                tile_position=(po, po),
                            )
                    if hp % 2 == 1:
                        # Y^T[c, g*256 + d*4+wq] = yw[c, g*256 + wq*64+d]
                        ytt = yt_pool.tile([128, 512], F16, tag="yt", name="ytt")
                        eng = (
                            nc.vector.tensor_copy
                            if (hp // 2) % 2
                            else nc.scalar.copy
                        )
                        eng(
                            ytt[:].rearrange("p (g b a) -> p g a b", g=2, a=4),
                            yw[hp // 2][:].rearrange(
                                "p (g a b) -> p g a b", g=2, a=4
                            ),
                        )
                        yt2[hp // 2] = ytt
                return f

            def u_op(th):
                def f():
                    po_m = [
                        ps_a.tile([128, 512], F32, tag="a", name=f"pom{i}")
                        for i in range(2)
                    ]
                    for cb in range(8):
                        lh = yt2[cb // 2][
                            :, (cb % 2) * 256 + th * 128 : (cb % 2) * 256 + th * 128 + 128
                        ]
                        for mi in range(2):
                            nc.tensor.matmul(
                                po_m[mi][:],
                                lh,
                                wo_sb[cb][:, mi * 512 : (mi + 1) * 512],
                                start=(cb == 0),
                                stop=(not with_bias and cb == 7),
                            )
                    if with_bias:
                        for mi in range(2):
                            nc.tensor.matmul(
                                po_m[mi][:],
                                ones[:, :],
                                bo_sb[:, mi * 512 : (mi + 1) * 512],
                                start=False,
                                stop=True,
                            )
                    ot = o_pool.tile([128, C], F32, tag="o", name="ot")
                    nc.vector.tensor_copy(ot[:, 0:512], po_m[0][:])
                    nc.scalar.copy(ot[:, 512:1024], po_m[1][:])
                    row = w * 256 + th * 128
                    nc.sync.dma_start(out.ap()[row : row + 128, :], ot[:])
                return f

            return [
                u_pt(0), u_pt(1), u_o(0), u_o(1), u_o(2), u_o(3),
                u_pt(2), u_pt(3), u_o(4), u_o(5), u_o(6), u_o(7),
                u_op(0), u_op(1),
            ]

        for r in range(NCHUNK + 2):
            qk = qkv_units(r) if r < NCHUNK else []
            win = []
            if 2 <= r < NWIN + 2:
                win += sphase_units(r - 2)
            if 3 <= r < NWIN + 3:
                win += phase2_units(r - 3)
            # S-phase first (its deps are ready), then phase2 of the
            # previous window
            for u in interleave(qk, win):
                u()

    nc.compile()
    return nc


_CACHE = {}
_LOCK = threading.Lock()


def _get_program(with_bias=False):
    key = f"nc_bias{with_bias}"
    with _LOCK:
        if key not in _CACHE:
            _CACHE[key] = build_program(with_bias=with_bias)
        return _CACHE[key]


def make_in_maps(x, W_qkv, b_qkv, W_out, b_out):
    x16 = np.asarray(x, dtype=np.float16)
    wqkv16 = np.asarray(W_qkv, dtype=np.float16)
    wout16 = np.asarray(W_out, dtype=np.float16)
    bqkv = np.asarray(b_qkv, dtype=np.float32)
    bout = np.asarray(b_out, dtype=np.float32)
    in_maps = []
    for cid in range(8):
        b, half = cid // 2, cid % 2
        t0 = half * NWIN * STEP
        in_maps.append(
            {
                "xs": np.ascontiguousarray(x16[b, t0 : t0 + TOK, :]),
                "wqkv": wqkv16,
                "bqkv": bqkv,
                "wout": wout16,
                "bout": bout,
            }
        )
    return in_maps


def kernel(x, W_qkv, b_qkv, W_out, b_out):
    with_bias = bool(np.any(b_qkv)) or bool(np.any(b_out))
    nc = _get_program(with_bias=with_bias)
    in_maps = make_in_maps(x, W_qkv, b_qkv, W_out, b_out)
    res = run_bass_kernel_spmd(nc, in_maps, core_ids=list(range(8)))
    out_full = np.empty((B, L, C), dtype=np.float32)
    for cid in range(8):
        b, half = cid // 2, cid % 2
        out_full[b, half * OUT_ROWS : (half + 1) * OUT_ROWS, :] = res.results[cid][
            "out"
        ]
    return out_full
